# revision 2
# baseline (speedup 1.0000x reference)
"""BiLSTM (B=64, T=512, D_IN=512, H=1024) on 8 TRN2 NeuronCores — v4.

Same H-sharded structure as the baseline kernel (core j owns a 128-wide
slice of H for all four gates, both directions; h @ W_hh gate matmuls with
h^T chunks stationary; per-step AllGather of h^T), with one change: the
forward and backward directions share a SINGLE AllGather per step instead
of one each. Both directions' h_new^T slices are transposed into one
[128, 2B] tile, gathered in one collective ([H, 2B]), and DMA'd back into
one fused h^T buffer — halving the number of per-step collectives, whose
~5us ncfw floor dominates the recurrence's critical path.
"""

import sys

if "/opt/trn_rl_repo" not in sys.path:
    sys.path.insert(0, "/opt/trn_rl_repo")

from contextlib import ExitStack

import numpy as np

B, T, D_IN, H, D_OUT = 64, 512, 512, 1024, 512
NC_N = 8
HJ = H // NC_N  # 128 — per-core H slice
GJ = 4 * HJ  # 512 — per-core gate columns (i|f|g|o, 128 each)
KD = D_IN // 128  # 4 k-chunks over D_IN
KH = H // 128  # 8 k-chunks over H


def build(t_steps=T):
    import concourse.mybir as mybir
    import concourse.tile as tile
    from concourse import bacc
    from concourse.masks import make_identity

    f32 = mybir.dt.float32
    f32r = mybir.dt.float32r
    AF = mybir.ActivationFunctionType

    nc = bacc.Bacc(None, target_bir_lowering=False, num_devices=NC_N)

    xT = nc.dram_tensor("xT", [t_steps, D_IN, B], f32r, kind="ExternalInput")
    wih = {}
    whh = {}
    bias = {}
    for d in "fb":
        wih[d] = nc.dram_tensor(f"wihT_{d}", [D_IN, GJ], f32r, kind="ExternalInput")
        whh[d] = nc.dram_tensor(f"whhT_{d}", [H, GJ], f32r, kind="ExternalInput")
        bias[d] = nc.dram_tensor(f"bias_{d}", [B, GJ], f32, kind="ExternalInput")
    wlin = nc.dram_tensor("wlinT", [2 * H, D_OUT], f32r, kind="ExternalInput")
    blin = nc.dram_tensor("blin", [B, D_OUT], f32, kind="ExternalInput")
    out = nc.dram_tensor("out", [B, D_OUT], f32, kind="ExternalOutput")

    with ExitStack() as ctx:
        tc = ctx.enter_context(tile.TileContext(nc))
        const = ctx.enter_context(tc.tile_pool(name="const", bufs=1))
        state = ctx.enter_context(tc.tile_pool(name="state", bufs=1))
        xpool = ctx.enter_context(tc.tile_pool(name="xpool", bufs=6))
        work = ctx.enter_context(tc.tile_pool(name="work", bufs=3))
        pg = ctx.enter_context(tc.tile_pool(name="pg", bufs=2, space="PSUM"))
        ptr = ctx.enter_context(tc.tile_pool(name="ptr", bufs=2, space="PSUM"))
        dram = ctx.enter_context(tc.tile_pool(name="dram", bufs=2, space="DRAM"))

        wih_sb = {}
        whh_sb = {}
        bias_sb = {}
        for d in "fb":
            wih_sb[d] = const.tile([128, KD, GJ], f32r, name=f"wih_sb_{d}")
            nc.sync.dma_start(
                wih_sb[d][:], wih[d][:].rearrange("(c p) n -> p c n", p=128)
            )
            whh_sb[d] = const.tile([128, KH, GJ], f32r, name=f"whh_sb_{d}")
            nc.sync.dma_start(
                whh_sb[d][:], whh[d][:].rearrange("(c p) n -> p c n", p=128)
            )
            bias_sb[d] = const.tile([B, GJ], f32, name=f"bias_sb_{d}")
            nc.sync.dma_start(bias_sb[d][:], bias[d][:])
        wlin_sb = const.tile([128, 2 * KH, D_OUT], f32r)
        nc.sync.dma_start(wlin_sb[:], wlin[:].rearrange("(c p) n -> p c n", p=128))
        blin_sb = const.tile([B, D_OUT], f32)
        nc.sync.dma_start(blin_sb[:], blin[:])
        ident = const.tile([B, B], f32)
        make_identity(nc, ident[:])

        # Fused gathered h^T state: [:, q, 0:B] = forward, [:, q, B:2B] = backward
        hT = state.tile([128, KH, 2 * B], f32r, name="hT")
        nc.vector.memset(hT[:].bitcast(f32), 0.0)
        c_st = {}
        for d in "fb":
            c_st[d] = state.tile([B, HJ], f32, name=f"c_{d}")
            nc.vector.memset(c_st[d][:], 0.0)

        for t in range(t_steps):
            tr_all = ptr.tile([HJ, 2 * B], f32, tag="tr_all", name=f"tr{t}")
            for d, t_eff, off in (("f", t, 0), ("b", t_steps - 1 - t, B)):
                xt = xpool.tile([128, KD, B], f32r, tag=f"xt_{d}", name=f"xt_{d}{t}")
                nc.sync.dma_start(
                    xt[:], xT[t_eff].rearrange("(c p) b -> p c b", p=128)
                )
                g_ps = pg.tile([B, GJ], f32, tag=f"g_ps_{d}", name=f"g_ps_{d}{t}")
                for k in range(KD):
                    nc.tensor.matmul(
                        g_ps[:],
                        xt[:, k, :],
                        wih_sb[d][:, k, :],
                        start=(k == 0),
                        stop=False,
                    )
                for k in range(KH):
                    nc.tensor.matmul(
                        g_ps[:],
                        hT[:, k, off : off + B],
                        whh_sb[d][:, k, :],
                        start=False,
                        stop=(k == KH - 1),
                    )
                pre = work.tile([B, GJ], f32, tag=f"pre_{d}", name=f"pre_{d}{t}")
                nc.vector.tensor_add(pre[:], g_ps[:], bias_sb[d][:])
                acts = work.tile([B, GJ], f32, tag=f"acts_{d}", name=f"acts_{d}{t}")
                nc.scalar.activation(acts[:, 0:HJ], pre[:, 0:HJ], AF.Sigmoid)
                nc.scalar.activation(
                    acts[:, HJ : 2 * HJ], pre[:, HJ : 2 * HJ], AF.Sigmoid
                )
                nc.scalar.activation(
                    acts[:, 2 * HJ : 3 * HJ], pre[:, 2 * HJ : 3 * HJ], AF.Tanh
                )
                nc.scalar.activation(
                    acts[:, 3 * HJ : 4 * HJ], pre[:, 3 * HJ : 4 * HJ], AF.Sigmoid
                )
                ig = work.tile([B, HJ], f32, tag=f"ig_{d}", name=f"ig_{d}{t}")
                fc = work.tile([B, HJ], f32, tag=f"fc_{d}", name=f"fc_{d}{t}")
                nc.vector.tensor_mul(ig[:], acts[:, 0:HJ], acts[:, 2 * HJ : 3 * HJ])
                nc.vector.tensor_mul(fc[:], acts[:, HJ : 2 * HJ], c_st[d][:])
                nc.vector.tensor_add(c_st[d][:], ig[:], fc[:])
                tnh = work.tile([B, HJ], f32, tag=f"tnh_{d}", name=f"tnh_{d}{t}")
                nc.scalar.activation(tnh[:], c_st[d][:], AF.Tanh)
                hnew = work.tile([B, HJ], f32, tag=f"hnew_{d}", name=f"hnew_{d}{t}")
                nc.vector.tensor_mul(hnew[:], acts[:, 3 * HJ : 4 * HJ], tnh[:])
                nc.tensor.matmul(
                    tr_all[:, off : off + B],
                    hnew[:],
                    ident[:],
                    is_transpose=True,
                    skip_group_check=True,
                )
            tr_sb = work.tile([HJ, 2 * B], f32, tag="tr_sb", name=f"trsb{t}")
            nc.vector.tensor_copy(tr_sb[:], tr_all[:])
            ag_i = dram.tile([HJ, 2 * B], f32r, tag="ag_i", name=f"agi{t}")
            ag_o = dram.tile(
                [H, 2 * B], f32r, tag="ag_o", name=f"ago{t}", addr_space="Shared"
            )
            nc.sync.dma_start(ag_i[:], tr_sb[:].bitcast(f32r))
            nc.gpsimd.collective_compute(
                "AllGather",
                mybir.AluOpType.bypass,
                replica_groups=[list(range(NC_N))],
                ins=[ag_i[:].opt()],
                outs=[ag_o[:].opt()],
            )
            nc.sync.dma_start(hT[:], ag_o[:].rearrange("(c p) b -> p c b", p=128))

        o_ps = pg.tile([B, D_OUT], f32, tag="o_ps", bufs=1)
        for k in range(KH):
            nc.tensor.matmul(
                o_ps[:],
                hT[:, k, 0:B],
                wlin_sb[:, k, :],
                start=(k == 0),
                stop=False,
            )
        for k in range(KH):
            nc.tensor.matmul(
                o_ps[:],
                hT[:, k, B : 2 * B],
                wlin_sb[:, KH + k, :],
                start=False,
                stop=(k == KH - 1),
            )
        o_sb = work.tile([B, D_OUT], f32, tag="o_sb")
        nc.vector.tensor_add(o_sb[:], o_ps[:], blin_sb[:])
        nc.sync.dma_start(out[:], o_sb[:])
    nc.compile()
    return nc


def make_in_maps(
    x, W_ih_f, W_hh_f, b_ih_f, b_hh_f, W_ih_b, W_hh_b, b_ih_b, b_hh_b, W_lin, b_lin
):
    xTs = np.ascontiguousarray(np.asarray(x, np.float32).transpose(1, 2, 0))
    W = {
        "f": (np.asarray(W_ih_f, np.float32), np.asarray(W_hh_f, np.float32),
              np.asarray(b_ih_f, np.float32) + np.asarray(b_hh_f, np.float32)),
        "b": (np.asarray(W_ih_b, np.float32), np.asarray(W_hh_b, np.float32),
              np.asarray(b_ih_b, np.float32) + np.asarray(b_hh_b, np.float32)),
    }
    wlinT = np.ascontiguousarray(np.asarray(W_lin, np.float32).T)  # [2H, D_OUT]
    blin_rep = np.broadcast_to(np.asarray(b_lin, np.float32), (B, D_OUT)).copy()
    in_maps = []
    for j in range(NC_N):
        m = {"xT": xTs, "wlinT": wlinT, "blin": blin_rep}
        cols = np.concatenate(
            [np.arange(g * H + j * HJ, g * H + (j + 1) * HJ) for g in range(4)]
        )
        for d in "fb":
            W_ih, W_hh, b_sum = W[d]
            m[f"wihT_{d}"] = np.ascontiguousarray(W_ih.T[:, cols])  # [D_IN, GJ]
            m[f"whhT_{d}"] = np.ascontiguousarray(W_hh.T[:, cols])  # [H, GJ]
            m[f"bias_{d}"] = np.broadcast_to(b_sum[cols], (B, GJ)).copy()
        in_maps.append(m)
    return in_maps


def kernel(**inputs) -> np.ndarray:
    from concourse.bass_utils import run_bass_kernel_spmd

    in_maps = make_in_maps(**inputs)
    nc = build(inputs["x"].shape[1])
    res = run_bass_kernel_spmd(nc, in_maps, core_ids=list(range(NC_N)))
    return res.results[0]["out"]
